# revision 41
# baseline (speedup 1.0000x reference)
"""MLA attention kernel for 8 TRN2 NeuronCores.

Sharding: core i handles batch b=i//4, heads h in [4*(i%4), 4*(i%4)+4).
Each head writes a disjoint 128-col slice of the output (the reference's
output einsum shares `h` between attention heads and output-channel
blocks), so no collective is needed: pure SPMD + host concat.

Math per core (batch b, 4 heads), equivalent to the reference modulo
fp reassociation:
  c_q  = x_b @ W_dq.T                 [T, 512]
  c_kv = x_b @ W_dkv.T                [T, 512]
  k_r  = rope(x_b @ W_kr.T)           [T, 64]   (shared by heads)
  per head h:
    q_h  = c_q @ A_h / sqrt(192)      [T, 128]  A_h = W_uq.reshape(512,16,128)[:,h,:]
    k_h  = c_kv @ B_h.T               [T, 128]  B_h = W_uk.reshape(16,128,512)[h]
    q_r  = rope(c_q @ W_qr_h.T)/sqrt  [T, 64]
    v_eff= W_uv.T @ W_o[h-block].T    [512, 128]
    v_h  = c_kv @ v_eff               [T, 128]
    S    = q_h k_h^T + q_r k_r^T  (causal)
    y_h  = softmax(S) @ v_h  ->  out[:, h*128:(h+1)*128]

On-chip layout: scores computed transposed S_T[s, t] so the exp output
P_T[s, t] feeds the PV matmul directly as the stationary operand
(contraction over s = partitions).  Row sums for softmax normalization
come from a ones-column appended to v_h (PV output col 128).  Scores
are O(1) here so exp needs no max-subtraction.  All inputs are
pre-arranged on the host into the exact SBUF tile layouts (partition-
major) so every DMA moves contiguous 16KB-per-partition runs.
"""

import sys

sys.path.insert(0, "/opt/trn_rl_repo")

import numpy as np
import ml_dtypes
from contextlib import ExitStack

import concourse.bass as bass
import concourse.bacc as bacc
import concourse.mybir as mybir
import concourse.tile as tile
from concourse.bass_utils import run_bass_kernel_spmd

B, T, C = 2, 2048, 2048
NH, HS = 16, 128
NLQ, NLKV, DHR = 512, 512, 64
HPC = 4  # heads per core
NCORES = 8
SCALE = 1.0 / float(np.sqrt(HS + DHR))

bf16 = ml_dtypes.bfloat16
F32 = mybir.dt.float32
B16 = mybir.dt.bfloat16
Copy = mybir.ActivationFunctionType.Copy
Exp = mybir.ActivationFunctionType.Exp
F8A = mybir.dt.float8e4

DR = mybir.MatmulPerfMode.DoubleRow
fp8 = ml_dtypes.float8_e4m3

NT = T // 512  # 4 t-chunks
NS = T // 128  # 16 s-tiles
PV_LAG = 3  # S/exp runs this many s-tiles ahead of PV


GROUPS = [[0, 1, 2, 3], [4, 5, 6, 7]]


def build():
    nc = bacc.Bacc("TRN2", target_bir_lowering=False, debug=False, num_devices=NCORES)

    # full x (absolute chunk order) for the replicated c_q compute; the
    # own T-chunk again as xpo for the distributed c_kv / k_r prologue
    # (c_kv and k_r are computed 1/4 per core and all-gathered across the
    # 4 cores of the same batch — their consumers run late enough to hide
    # the gather, while c_q's consumers don't, so c_q stays replicated)
    xp = nc.dram_tensor("xp", [128, NT, 16, 512], B16, kind="ExternalInput")
    xpo = nc.dram_tensor("xpo", [128, 16, 512], B16, kind="ExternalInput")
    cosk = nc.dram_tensor("cosk", [DHR // 2, 512], B16, kind="ExternalInput")
    sink = nc.dram_tensor("sink", [DHR // 2, 512], B16, kind="ExternalInput")
    wdq = nc.dram_tensor("wdq", [128, 16, NLQ], B16, kind="ExternalInput")
    wdkv = nc.dram_tensor("wdkv", [128, 16, NLKV], B16, kind="ExternalInput")
    wkr = nc.dram_tensor("wkr", [128, 16, DHR], B16, kind="ExternalInput")
    wqr = nc.dram_tensor("wqr", [HPC, 128, 4, DHR], B16, kind="ExternalInput")
    A = nc.dram_tensor("A", [HPC, 128, 4, HS], B16, kind="ExternalInput")
    BT = nc.dram_tensor("BT", [HPC, 128, 4, HS], B16, kind="ExternalInput")
    wuv = nc.dram_tensor("wuv", [128, 16, NLKV], B16, kind="ExternalInput")
    woT = nc.dram_tensor("woT", [128, 16, HPC * HS], B16, kind="ExternalInput")
    cosT = nc.dram_tensor("cosT", [DHR // 2, T], B16, kind="ExternalInput")
    sinT = nc.dram_tensor("sinT", [DHR // 2, T], B16, kind="ExternalInput")
    masks = nc.dram_tensor("masks", [128, 896], B16, kind="ExternalInput")
    out = nc.dram_tensor("out", [T, HPC * HS], F32, kind="ExternalOutput")

    with tile.TileContext(nc) as tc, ExitStack() as ctx:
        wpool = ctx.enter_context(tc.tile_pool(name="wpool", bufs=1))
        xpool = ctx.enter_context(tc.tile_pool(name="xpool", bufs=2))
        cpool = ctx.enter_context(tc.tile_pool(name="cpool", bufs=1))
        hwpool = ctx.enter_context(tc.tile_pool(name="hwpool", bufs=2))
        hpool = ctx.enter_context(tc.tile_pool(name="hpool", bufs=2))
        ppool = ctx.enter_context(tc.tile_pool(name="ppool", bufs=5))
        tpool = ctx.enter_context(tc.tile_pool(name="tpool", bufs=2))
        opool = ctx.enter_context(tc.tile_pool(name="opool", bufs=2))
        psW = ctx.enter_context(tc.tile_pool(name="psW", bufs=4, space="PSUM"))
        psY = ctx.enter_context(tc.tile_pool(name="psY", bufs=4, space="PSUM"))

        # ---- persistent weight loads, issued in need-order (per-queue
        # FIFO makes issue order the DMA priority), split into many
        # dma_starts so they spread across queues ----
        wkr_sb = wpool.tile([128, 16, DHR], B16)
        for g in range(2):
            nc.sync.dma_start(
                out=wkr_sb[:, g * 8 : (g + 1) * 8, :],
                in_=wkr.ap()[:, g * 8 : (g + 1) * 8],
            )
        cosk_sb = wpool.tile([32, 512], B16)
        nc.sync.dma_start(out=cosk_sb[:], in_=cosk.ap())
        sink_sb = wpool.tile([32, 512], B16)
        nc.sync.dma_start(out=sink_sb[:], in_=sink.ap())

        # ---- local prologue: own T-chunk of c_q, c_kv, k_r ----
        # full-T tensors (peer chunks arrive via AllGather)
        cq_sb = cpool.tile([128, 4, T], B16)
        ckv_sb = cpool.tile([128, 4, T], B16)
        # shared fp8 rope-K operand: rows 0:64 = fp8(k_r), rows 64:128 =
        # fp8(4*(k_r - fp8(k_r))) (the hi/lo split cancels k_r's fp8
        # quantization error in the S matmul)
        krh_sb = cpool.tile([128, T], F8A)

        dram = ctx.enter_context(tc.tile_pool(name="dram", bufs=1, space="DRAM"))
        krh_bi = dram.tile([128, 512], F8A)
        krh_bo = dram.tile([4, 128, 512], F8A)
        ckv_bi = dram.tile([128, 4, 512], B16)
        ckv_bo = dram.tile([4, 128, 4, 512], B16)

        # tiny warmup collective: gets CC channel setup going while the
        # prologue DMAs/matmuls run
        warm_bi = dram.tile([128, 16], F8A)
        warm_bo = dram.tile([4, 128, 16], F8A)
        warm_sb = tpool.tile([128, 16], F8A, tag="warm", bufs=1)
        nc.gpsimd.memset(warm_sb[:], 0.0)
        nc.gpsimd.dma_start(out=warm_bi[:], in_=warm_sb[:])
        nc.gpsimd.collective_compute(
            "AllGather",
            mybir.AluOpType.bypass,
            replica_groups=GROUPS,
            ins=[warm_bi[:].opt()],
            outs=[warm_bo[:].opt()],
        )

        dma_engs = [nc.scalar, nc.sync, nc.gpsimd, nc.scalar]
        halves = []
        for hf in range(2):
            xh = xpool.tile([128, 8, 512], B16, tag="xslo", bufs=2, name=f"xsl_{hf}")
            for g in range(4):
                for u in range(2):
                    c0 = hf * 8 + g * 2 + u
                    dma_engs[g].dma_start(
                        out=xh[:, g * 2 + u : g * 2 + u + 1, :],
                        in_=xpo.ap()[:, c0 : c0 + 1],
                    )
            halves.append(xh)

        wdq_sb = wpool.tile([128, 16, NLQ], B16)
        for g in range(8):
            nc.sync.dma_start(
                out=wdq_sb[:, g * 2 : (g + 1) * 2, :],
                in_=wdq.ap()[:, g * 2 : (g + 1) * 2],
            )
        # wdkv's buffer is reused for wuv: wdkv's last read is the c_kv-own
        # matmuls, well before veff (emitted after the c_q phase) needs wuv
        wdkv_sb = wpool.tile([128, 16, NLKV], B16, tag="wkv", bufs=1, name="wdkv")
        for g in range(8):
            nc.scalar.dma_start(
                out=wdkv_sb[:, g * 2 : (g + 1) * 2, :],
                in_=wdkv.ap()[:, g * 2 : (g + 1) * 2],
            )
        cos_sb = wpool.tile([32, T], B16)
        nc.sync.dma_start(out=cos_sb[:], in_=cosT.ap())
        sin_sb = wpool.tile([32, T], B16)
        nc.sync.dma_start(out=sin_sb[:], in_=sinT.ap())

        # per-head projection weights (needed from the q-projection phase)
        a_sbs, wqr_sbs, bt_sbs = [], [], []
        for h in range(HPC):
            a_sb = hwpool.tile([128, 4, HS], B16, tag="a_sb", bufs=4, name=f"a{h}")
            nc.sync.dma_start(out=a_sb[:], in_=A.ap()[h])
            wqr_sb = hwpool.tile(
                [128, 4, DHR], B16, tag="wqr_sb", bufs=4, name=f"wq{h}"
            )
            nc.sync.dma_start(out=wqr_sb[:], in_=wqr.ap()[h])
            bt_sb = hwpool.tile([128, 4, HS], B16, tag="bt_sb", bufs=4, name=f"bt{h}")
            nc.scalar.dma_start(out=bt_sb[:], in_=BT.ap()[h])
            a_sbs.append(a_sb)
            wqr_sbs.append(wqr_sb)
            bt_sbs.append(bt_sb)

        def xsl_at(ct):
            return halves[ct // 8][:, ct % 8, :]

        # k_r chunk first (its gather is tiny and unblocks kcat early).
        # The hi/lo staging lives in krh_sb's chunk-0 region: the bounce
        # DMA reads it before the gather-backs overwrite all chunks.
        kr_loc = cpool.tile([64, 512], B16)
        ps = psW.tile([128, 512], F32, tag="psw")
        for ct in range(16):
            nc.tensor.matmul(
                ps[0:64, :],
                wkr_sb[:, ct, :],
                xsl_at(ct),
                start=(ct == 0),
                stop=(ct == 15),
            )
        _rope(
            nc, tpool, ps, kr_loc[0:32, :], kr_loc[32:64, :], cosk_sb[:], sink_sb[:]
        )
        nc.scalar.activation(krh_sb[0:64, 0:512], kr_loc[:], Copy)
        lo_bf = tpool.tile([64, 512], B16, tag="krlo", bufs=1)
        nc.vector.tensor_sub(lo_bf[:], kr_loc[:], krh_sb[0:64, 0:512])
        nc.scalar.activation(krh_sb[64:128, 0:512], lo_bf[:], Copy, scale=4.0)
        nc.gpsimd.dma_start(out=krh_bi[:], in_=krh_sb[:, 0:512])
        nc.gpsimd.collective_compute(
            "AllGather",
            mybir.AluOpType.bypass,
            replica_groups=GROUPS,
            ins=[krh_bi[:].opt()],
            outs=[krh_bo[:].opt()],
        )

        # c_kv chunk (own quarter, gathered; its consumers run late).
        # Staged in ckv_sb's chunk-0 region; gather-backs overwrite it.
        for kt in range(4):
            ps = psW.tile([128, 512], F32, tag="psw")
            for ct in range(16):
                nc.tensor.matmul(
                    ps[:],
                    wdkv_sb[:, ct, kt * 128 : (kt + 1) * 128],
                    xsl_at(ct),
                    start=(ct == 0),
                    stop=(ct == 15),
                )
            nc.vector.tensor_copy(ckv_sb[:, kt, 0:512], ps[:])
        nc.gpsimd.dma_start(out=ckv_bi[:], in_=ckv_sb[:, :, 0:512])
        nc.gpsimd.collective_compute(
            "AllGather",
            mybir.AluOpType.bypass,
            replica_groups=GROUPS,
            ins=[ckv_bi[:].opt()],
            outs=[ckv_bo[:].opt()],
        )

        # mid-needed big weights: wuv reuses wdkv's buffer (released just
        # above); issued here so the buffer-reuse wait doesn't block
        # earlier scalar work
        wuv_sb = wpool.tile([128, 16, NLKV], B16, tag="wkv", bufs=1, name="wuv")
        for g in range(4):
            nc.scalar.dma_start(
                out=wuv_sb[:, g * 4 : (g + 1) * 4, :],
                in_=wuv.ap()[:, g * 4 : (g + 1) * 4],
            )
        woh_sb = wpool.tile([128, 16, HPC * HS], B16)
        for g in range(4):
            nc.scalar.dma_start(
                out=woh_sb[:, g * 4 : (g + 1) * 4, :],
                in_=woT.ap()[:, g * 4 : (g + 1) * 4],
            )
        mask_sb = wpool.tile([128, 896], B16)
        nc.scalar.dma_start(out=mask_sb[:], in_=masks.ap())

        # gather-backs (gpsimd: it is the comms engine and has nothing
        # else pending, so its post-AG stall is free)
        for j in range(NT):
            nc.gpsimd.dma_start(
                out=krh_sb[:, j * 512 : (j + 1) * 512], in_=krh_bo[:][j]
            )
        for j in range(NT):
            nc.gpsimd.dma_start(
                out=ckv_sb[:, :, j * 512 : (j + 1) * 512], in_=ckv_bo[:][j]
            )

        # ---- replicated c_q over all chunks (fills the gather/skew
        # window; q projections can then start without waiting on comms)
        for c in range(NT):
            t0 = c * 512
            xf = []
            for q4 in range(4):
                xt = xpool.tile(
                    [128, 4, 512], B16, tag="xslf", bufs=5, name=f"xf_{c}_{q4}"
                )
                (nc.scalar if q4 % 2 == 0 else nc.sync).dma_start(
                    out=xt[:], in_=xp.ap()[:, c, q4 * 4 : (q4 + 1) * 4]
                )
                xf.append(xt)
            for qt in range(4):
                ps = psW.tile([128, 512], F32, tag="psw")
                for ct in range(16):
                    nc.tensor.matmul(
                        ps[:],
                        wdq_sb[:, ct, qt * 128 : (qt + 1) * 128],
                        xf[ct // 4][:, ct % 4, :],
                        start=(ct == 0),
                        stop=(ct == 15),
                    )
                nc.scalar.activation(cq_sb[:, qt, t0 : t0 + 512], ps[:], Copy)

        # ---- shared V phase: all 4 heads at once (N=512 matmuls) ----
        # veff_all[k, (h,d)] = sum_c W_uv[c,k] * W_o[h-block].T[c,d]
        veff_sb = cpool.tile([128, 4, HPC * HS], B16)
        for kt in range(4):
            ps = psW.tile([128, 512], F32, tag="psw")
            for ct in range(16):
                nc.tensor.matmul(
                    ps[:],
                    wuv_sb[:, ct, kt * 128 : (kt + 1) * 128],
                    woh_sb[:, ct, :],
                    start=(ct == 0),
                    stop=(ct == 15),
                )
            nc.vector.tensor_copy(veff_sb[:, kt, :], ps[:])

        # ---- Q projections for all heads ----
        qcats = []
        for h in range(HPC):
            # fp8 DoubleRow operands: slot 0 = content (128d), slot 1 =
            # rope.  K side: (kr_hi | 4*kr_lo); Q side: (qr | qr/4) so the
            # rope product is qr*kr_hi + (qr/4)*4*kr_lo = qr*kr exactly
            # in kr.
            qcat_sb = hpool.tile([128, 2, T], F8A, tag="qh", bufs=4, name=f"qcat{h}")
            for j in range(NT):
                t0 = j * 512
                ps = psW.tile([128, 512], F32, tag="psw")
                for qt in range(4):
                    nc.tensor.matmul(
                        ps[:],
                        a_sbs[h][:, qt, :],
                        cq_sb[:, qt, t0 : t0 + 512],
                        start=(qt == 0),
                        stop=(qt == 3),
                    )
                nc.scalar.activation(qcat_sb[:, 0, t0 : t0 + 512], ps[:], Copy)
                ps = psW.tile([128, 512], F32, tag="psw")
                for qt in range(4):
                    nc.tensor.matmul(
                        ps[0:64, :],
                        wqr_sbs[h][:, qt, :],
                        cq_sb[:, qt, t0 : t0 + 512],
                        start=(qt == 0),
                        stop=(qt == 3),
                    )
                _rope(
                    nc,
                    tpool,
                    ps,
                    qcat_sb[0:32, 1, t0 : t0 + 512],
                    qcat_sb[32:64, 1, t0 : t0 + 512],
                    cos_sb[:, t0 : t0 + 512],
                    sin_sb[:, t0 : t0 + 512],
                )
                nc.scalar.activation(
                    qcat_sb[64:128, 1, t0 : t0 + 512],
                    qcat_sb[0:64, 1, t0 : t0 + 512],
                    Copy,
                    scale=0.25,
                )
            qcats.append(qcat_sb)

        # v_aug[s, (h, d|1)]: v for all heads + ones column per head
        vaug_sb = cpool.tile([128, NS, HPC, 129], B16)
        for st in range(NS):
            ps = psW.tile([128, 512], F32, tag="psw")
            for kt in range(4):
                nc.tensor.matmul(
                    ps[:],
                    ckv_sb[:, kt, st * 128 : (st + 1) * 128],
                    veff_sb[:, kt, :],
                    start=(kt == 0),
                    stop=(kt == 3),
                )
            nc.vector.tensor_copy(
                vaug_sb[:, st, :, 0:128],
                ps[:].rearrange("p (h d) -> p h d", h=HPC),
            )
            nc.vector.memset(vaug_sb[:, st, :, 128:129], 1.0)

        # ---- per-head: K projection + attention ----
        for h in range(HPC):
            qcat_sb = qcats[h]
            kcat_sb = hpool.tile([128, 2, T], F8A, tag="kh")
            for j in range(NT):
                t0 = j * 512
                nc.vector.tensor_copy(
                    kcat_sb[:, 1, t0 : t0 + 512], krh_sb[:, t0 : t0 + 512]
                )
                ps = psW.tile([128, 512], F32, tag="psw")
                for kt in range(4):
                    nc.tensor.matmul(
                        ps[:],
                        bt_sbs[h][:, kt, :],
                        ckv_sb[:, kt, t0 : t0 + 512],
                        start=(kt == 0),
                        stop=(kt == 3),
                    )
                nc.vector.tensor_copy(kcat_sb[:, 0, t0 : t0 + 512], ps[:])

            # attention: for each t-chunk, accumulate over causal s-tiles.
            # S/exp runs PV_LAG s-tiles ahead of PV so the PE stream does
            # not stall on psY slot release at chunk boundaries.
            for j in range(NT):
                t0 = j * 512
                ys = [
                    psY.tile([128, 132], F32, tag="psy", name=f"psy_{h}_{j}_{m}")
                    for m in range(4)
                ]
                n_st = 4 * j + 4
                pts = {}

                def s_exp(i, h=h, j=j, t0=t0):
                    ss = psW.tile([128, 512], F32, tag="psw")
                    nc.tensor.matmul(
                        ss[:],
                        kcat_sb[:, :, i * 128 : (i + 1) * 128],
                        qcat_sb[:, :, t0 : t0 + 512],
                        start=True,
                        stop=True,
                        perf_mode=DR,
                    )
                    pt = ppool.tile([128, 512], B16, tag="pt", name=f"pt_{h}_{j}_{i}")
                    m2 = i - 4 * j
                    if m2 <= 0:
                        nc.scalar.activation(pt[:], ss[:], Exp, scale=SCALE)
                        if m2 == 0:
                            nc.vector.tensor_mul(
                                pt[:], pt[:], mask_sb[:, 384:896]
                            )
                    else:
                        # band tile: PV only reads cols >= 128*m2; exp/mask
                        # just that suffix (the prefix is never consumed)
                        lo = 128 * m2
                        nc.scalar.activation(
                            pt[:, lo:512], ss[:, lo:512], Exp, scale=SCALE
                        )
                        nc.vector.tensor_mul(
                            pt[:, lo:512],
                            pt[:, lo:512],
                            mask_sb[:, 384 : 896 - lo],
                        )
                    pts[i] = pt

                def pv(i, ys=ys, pts=pts, j=j):
                    mm0 = max(0, i - 4 * j)
                    for m in range(mm0, 4):
                        nc.tensor.matmul(
                            ys[m][:, 0:129],
                            pts[i][:, m * 128 : (m + 1) * 128],
                            vaug_sb[:, i, h, 0:129],
                            start=(i == 0),
                            stop=(i == 4 * j + m),
                        )

                for i in range(n_st):
                    s_exp(i)
                    if i >= PV_LAG:
                        pv(i - PV_LAG)
                for i in range(max(0, n_st - PV_LAG), n_st):
                    pv(i)

                for m in range(4):
                    recip = tpool.tile([128, 1], F32, tag="recip")
                    nc.vector.reciprocal(recip[:], ys[m][:, 128:129])
                    o_sb = opool.tile([128, HS], F32, tag="o_sb")
                    nc.vector.tensor_scalar_mul(o_sb[:], ys[m][:, 0:128], recip[:])
                    nc.scalar.dma_start(
                        out=out.ap()[
                            t0 + m * 128 : t0 + (m + 1) * 128,
                            h * HS : (h + 1) * HS,
                        ],
                        in_=o_sb[:],
                    )

    nc.compile()
    return nc


def _rope(nc, tpool, ps, dst_re, dst_im, cs, sn):
    """ps[0:64, :512] holds the projected (re|im col-permuted) vectors.
    Write roped values into dst_re (partitions 0:32 of the target) and
    dst_im (partitions 32:64).  DVE tensor_tensor requires all operands
    at the same start partition, so the im half is staged through base-0
    tiles with ACT copies (ACT allows cross-base)."""
    im_sb = tpool.tile([32, 512], B16, tag="imsrc", bufs=1)
    nc.scalar.activation(im_sb[:], ps[32:64, :], Copy)
    t1 = tpool.tile([32, 512], B16, tag="ropet1", bufs=2)
    t2 = tpool.tile([32, 512], B16, tag="ropet2", bufs=2)
    nc.vector.tensor_mul(t1[:], ps[0:32, :], cs)
    nc.vector.tensor_mul(t2[:], im_sb[:], sn)
    nc.vector.tensor_sub(dst_re, t1[:], t2[:])
    t3 = tpool.tile([32, 512], B16, tag="ropet1", bufs=2, name="t3")
    t4 = tpool.tile([32, 512], B16, tag="ropet2", bufs=2, name="t4")
    nc.vector.tensor_mul(t3[:], ps[0:32, :], sn)
    nc.vector.tensor_mul(t4[:], im_sb[:], cs)
    im_ro = tpool.tile([32, 512], B16, tag="imrope", bufs=1)
    nc.vector.tensor_add(im_ro[:], t3[:], t4[:])
    nc.scalar.activation(dst_im, im_ro[:], Copy)


_NC_CACHE = {}


def _get_nc():
    if "nc" not in _NC_CACHE:
        _NC_CACHE["nc"] = build()
    return _NC_CACHE["nc"]


def _part_major(a, pt=128):
    """[pt*n, ...] -> [128, n, ...] partition-major contiguous."""
    n = a.shape[0] // pt
    return np.ascontiguousarray(
        a.reshape(n, pt, *a.shape[1:]).transpose(1, 0, *range(2, a.ndim + 1))
    )


def _prep_in_maps(x, cos, sin, W_dq, W_uq, W_dkv, W_uk, W_uv, W_qr, W_kr, W_o):
    perm = np.concatenate([np.arange(0, DHR, 2), np.arange(1, DHR, 2)])

    shared = {
        "wdq": _part_major(np.ascontiguousarray(W_dq.T).astype(bf16)),
        "wdkv": _part_major(np.ascontiguousarray(W_dkv.T).astype(bf16)),
        "wkr": _part_major(np.ascontiguousarray(W_kr.T[:, perm]).astype(bf16)),
        "wuv": _part_major(np.ascontiguousarray(W_uv).astype(bf16)),
        "cosT": np.ascontiguousarray(cos.T).astype(bf16),
        "sinT": np.ascontiguousarray(sin.T).astype(bf16),
    }
    p = np.arange(128)[:, None]
    u = np.arange(896)[None, :]
    shared["masks"] = np.ascontiguousarray((p <= u - 384).astype(bf16))  # [128, 896]

    # x[b] -> xp[p, j, ct, f] = x[b][j*512+f, ct*128+p]
    xps = []
    for b in range(B):
        xb = np.asarray(x[b]).astype(bf16)  # [T, C]
        xps.append(
            np.ascontiguousarray(xb.reshape(NT, 512, 16, 128).transpose(3, 0, 2, 1))
        )

    A_full = np.asarray(W_uq).reshape(NLQ, NH, HS)
    B_full = np.asarray(W_uk).reshape(NH, HS, NLKV)

    head_maps = []
    for g in range(4):
        hs = [4 * g + i for i in range(HPC)]
        # SCALE is applied in the Exp activation on-chip (keeps the fp8
        # q/k operands in a healthy range)
        A_np = np.stack([_part_major(A_full[:, h, :].astype(bf16)) for h in hs])
        BT_np = np.stack([_part_major(np.ascontiguousarray(B_full[h].T).astype(bf16)) for h in hs])
        wqr_np = np.stack(
            [
                _part_major(
                    np.ascontiguousarray(
                        W_qr[h * DHR : (h + 1) * DHR, :].T[:, perm]
                    ).astype(bf16)
                )
                for h in hs
            ]
        )
        # [128(c p), 16(ct), HPC*HS] with free = (h, d)
        woT_np = np.stack(
            [
                _part_major(np.ascontiguousarray(W_o[h * HS : (h + 1) * HS, :].T).astype(bf16))
                for h in hs
            ],
            axis=2,
        ).reshape(128, 16, HPC * HS)
        head_maps.append(
            {
                "A": np.ascontiguousarray(A_np),
                "BT": np.ascontiguousarray(BT_np),
                "wqr": np.ascontiguousarray(wqr_np),
                "woT": np.ascontiguousarray(woT_np),
            }
        )

    cosT_np = shared["cosT"]
    sinT_np = shared["sinT"]
    in_maps = []
    for core in range(NCORES):
        b, g = core // 4, core % 4
        im = dict(shared)
        # full x for the replicated c_q compute; own T-chunk again for the
        # distributed c_kv/k_r prologue, plus the matching cos/sin slice
        # for the local k_r rope
        im["xp"] = xps[b]
        im["xpo"] = np.ascontiguousarray(xps[b][:, g])
        im["cosk"] = np.ascontiguousarray(cosT_np[:, g * 512 : (g + 1) * 512])
        im["sink"] = np.ascontiguousarray(sinT_np[:, g * 512 : (g + 1) * 512])
        im.update(head_maps[g])
        in_maps.append(im)
    return in_maps


def kernel_run(inputs, trace=False, trace_kwargs=None):
    nc = _get_nc()
    in_maps = _prep_in_maps(**{k: np.asarray(v) for k, v in inputs.items()})
    res = run_bass_kernel_spmd(
        nc,
        in_maps,
        core_ids=list(range(NCORES)),
        trace=trace,
        **(trace_kwargs or {}),
    )
    y = np.empty((B, T, C), np.float32)
    for core in range(NCORES):
        b, g = core // 4, core % 4
        y[b][:, g * 512 : (g + 1) * 512] = res.results[core]["out"]
    return y, res


def kernel(**inputs):
    y, _ = kernel_run(inputs)
    return y



# revision 52
# speedup vs baseline: 1.2069x; 1.2069x over previous
"""MLA attention kernel for 8 TRN2 NeuronCores.

Sharding: core i handles batch b=i//4, heads h in [4*(i%4), 4*(i%4)+4).
Each head writes a disjoint 128-col slice of the output (the reference's
output einsum shares `h` between attention heads and output-channel
blocks), so no collective is needed: pure SPMD + host concat.

Math per core (batch b, 4 heads), equivalent to the reference modulo
fp reassociation:
  c_q  = x_b @ W_dq.T                 [T, 512]
  c_kv = x_b @ W_dkv.T                [T, 512]
  k_r  = rope(x_b @ W_kr.T)           [T, 64]   (shared by heads)
  per head h:
    q_h  = c_q @ A_h / sqrt(192)      [T, 128]  A_h = W_uq.reshape(512,16,128)[:,h,:]
    k_h  = c_kv @ B_h.T               [T, 128]  B_h = W_uk.reshape(16,128,512)[h]
    q_r  = rope(c_q @ W_qr_h.T)/sqrt  [T, 64]
    v_eff= W_uv.T @ W_o[h-block].T    [512, 128]
    v_h  = c_kv @ v_eff               [T, 128]
    S    = q_h k_h^T + q_r k_r^T  (causal)
    y_h  = softmax(S) @ v_h  ->  out[:, h*128:(h+1)*128]

On-chip layout: scores computed transposed S_T[s, t] so the exp output
P_T[s, t] feeds the PV matmul directly as the stationary operand
(contraction over s = partitions).  Row sums for softmax normalization
come from a ones-column appended to v_h (PV output col 128).  Scores
are O(1) here so exp needs no max-subtraction.  All inputs are
pre-arranged on the host into the exact SBUF tile layouts (partition-
major) so every DMA moves contiguous 16KB-per-partition runs.
"""

import sys

sys.path.insert(0, "/opt/trn_rl_repo")

import numpy as np
import ml_dtypes
from contextlib import ExitStack

import concourse.bass as bass
import concourse.bacc as bacc
import concourse.mybir as mybir
import concourse.tile as tile
from concourse.bass_utils import run_bass_kernel_spmd

B, T, C = 2, 2048, 2048
NH, HS = 16, 128
NLQ, NLKV, DHR = 512, 512, 64
HPC = 4  # heads per core
NCORES = 8
SCALE = 1.0 / float(np.sqrt(HS + DHR))

bf16 = ml_dtypes.bfloat16
F32 = mybir.dt.float32
B16 = mybir.dt.bfloat16
Copy = mybir.ActivationFunctionType.Copy
Exp = mybir.ActivationFunctionType.Exp
F8A = mybir.dt.float8e4

DR = mybir.MatmulPerfMode.DoubleRow
fp8 = ml_dtypes.float8_e4m3

NT = T // 512  # 4 t-chunks
NS = T // 128  # 16 s-tiles
PV_LAG = 3  # S/exp runs this many s-tiles ahead of PV


GROUPS = [[0, 1, 2, 3], [4, 5, 6, 7]]


def build():
    nc = bacc.Bacc("TRN2", target_bir_lowering=False, debug=False, num_devices=NCORES)

    # own T-chunk of x only: c_q, c_kv and k_r are computed 1/4 per core
    # and all-gathered across the 4 cores of the same batch
    xpo = nc.dram_tensor("xpo", [128, 16, 512], B16, kind="ExternalInput")
    cosk = nc.dram_tensor("cosk", [DHR // 2, 512], B16, kind="ExternalInput")
    sink = nc.dram_tensor("sink", [DHR // 2, 512], B16, kind="ExternalInput")
    wdq = nc.dram_tensor("wdq", [128, 16, NLQ], B16, kind="ExternalInput")
    wdkv = nc.dram_tensor("wdkv", [128, 16, NLKV], B16, kind="ExternalInput")
    wkr = nc.dram_tensor("wkr", [128, 16, DHR], B16, kind="ExternalInput")
    # head-PAIR packed W_qr: [pair, 128(q p), 4(qt), 2*DHR] — both heads'
    # rope projections come out of one matmul (partitions 0:64 / 64:128)
    wqr = nc.dram_tensor("wqr", [HPC // 2, 128, 4, 2 * DHR], B16, kind="ExternalInput")
    A = nc.dram_tensor("A", [HPC, 128, 4, HS], B16, kind="ExternalInput")
    BT = nc.dram_tensor("BT", [HPC, 128, 4, HS], B16, kind="ExternalInput")
    wuv = nc.dram_tensor("wuv", [128, 16, NLKV], B16, kind="ExternalInput")
    woT = nc.dram_tensor("woT", [128, 16, HPC * HS], B16, kind="ExternalInput")
    cosT = nc.dram_tensor("cosT", [DHR // 2, T], B16, kind="ExternalInput")
    sinT = nc.dram_tensor("sinT", [DHR // 2, T], B16, kind="ExternalInput")
    masks = nc.dram_tensor("masks", [128, 896], B16, kind="ExternalInput")
    out = nc.dram_tensor("out", [T, HPC * HS], F32, kind="ExternalOutput")

    with tile.TileContext(nc) as tc, ExitStack() as ctx:
        wpool = ctx.enter_context(tc.tile_pool(name="wpool", bufs=1))
        xpool = ctx.enter_context(tc.tile_pool(name="xpool", bufs=2))
        cpool = ctx.enter_context(tc.tile_pool(name="cpool", bufs=1))
        hwpool = ctx.enter_context(tc.tile_pool(name="hwpool", bufs=2))
        hpool = ctx.enter_context(tc.tile_pool(name="hpool", bufs=2))
        ppool = ctx.enter_context(tc.tile_pool(name="ppool", bufs=5))
        tpool = ctx.enter_context(tc.tile_pool(name="tpool", bufs=2))
        opool = ctx.enter_context(tc.tile_pool(name="opool", bufs=2))
        psW = ctx.enter_context(tc.tile_pool(name="psW", bufs=4, space="PSUM"))
        psY = ctx.enter_context(tc.tile_pool(name="psY", bufs=4, space="PSUM"))

        # ---- persistent weight loads, issued in need-order (per-queue
        # FIFO makes issue order the DMA priority), split into many
        # dma_starts so they spread across queues ----
        wkr_sb = wpool.tile([128, 16, DHR], B16)
        for g in range(2):
            nc.sync.dma_start(
                out=wkr_sb[:, g * 8 : (g + 1) * 8, :],
                in_=wkr.ap()[:, g * 8 : (g + 1) * 8],
            )
        cosk_sb = wpool.tile([32, 512], B16)
        nc.sync.dma_start(out=cosk_sb[:], in_=cosk.ap())
        sink_sb = wpool.tile([32, 512], B16)
        nc.sync.dma_start(out=sink_sb[:], in_=sink.ap())

        # ---- local prologue: own T-chunk of c_q, c_kv, k_r ----
        # full-T tensors (peer chunks arrive via AllGather)
        cq_sb = cpool.tile([128, 4, T], B16)
        ckv_sb = cpool.tile([128, 4, T], B16)
        # shared fp8 rope-K operand: rows 0:64 = fp8(k_r), rows 64:128 =
        # fp8(4*(k_r - fp8(k_r))) (the hi/lo split cancels k_r's fp8
        # quantization error in the S matmul)
        krh_sb = cpool.tile([128, T], F8A)

        dram = ctx.enter_context(tc.tile_pool(name="dram", bufs=1, space="DRAM"))
        krh_bi = dram.tile([128, 512], F8A)
        krh_bo = dram.tile([4, 128, 512], F8A)
        cq_bi = dram.tile([128, 4, 512], B16)
        cq_bo = dram.tile([4, 128, 4, 512], B16)
        ckv_bi = dram.tile([128, 4, 512], B16)
        ckv_bo = dram.tile([4, 128, 4, 512], B16)

        # tiny warmup collective: gets CC channel setup going while the
        # prologue DMAs/matmuls run
        warm_bi = dram.tile([128, 16], F8A)
        warm_bo = dram.tile([4, 128, 16], F8A)
        warm_sb = tpool.tile([128, 16], F8A, tag="warm", bufs=1)
        nc.gpsimd.memset(warm_sb[:], 0.0)
        nc.gpsimd.dma_start(out=warm_bi[:], in_=warm_sb[:])
        nc.gpsimd.collective_compute(
            "AllGather",
            mybir.AluOpType.bypass,
            replica_groups=GROUPS,
            ins=[warm_bi[:].opt()],
            outs=[warm_bo[:].opt()],
        )

        dma_engs = [nc.scalar, nc.sync, nc.sync, nc.scalar]
        halves = []
        for hf in range(2):
            xh = xpool.tile([128, 8, 512], B16, tag="xslo", bufs=2, name=f"xsl_{hf}")
            for g in range(4):
                for u in range(2):
                    c0 = hf * 8 + g * 2 + u
                    dma_engs[g].dma_start(
                        out=xh[:, g * 2 + u : g * 2 + u + 1, :],
                        in_=xpo.ap()[:, c0 : c0 + 1],
                    )
            halves.append(xh)

        wdq_sb = wpool.tile([128, 16, NLQ], B16)
        for g in range(8):
            nc.sync.dma_start(
                out=wdq_sb[:, g * 2 : (g + 1) * 2, :],
                in_=wdq.ap()[:, g * 2 : (g + 1) * 2],
            )
        # wdkv's buffer is reused for wuv: wdkv's last read is the c_kv-own
        # matmuls, well before veff (emitted after the c_q phase) needs wuv
        wdkv_sb = wpool.tile([128, 16, NLKV], B16, tag="wkv", bufs=1, name="wdkv")
        for g in range(8):
            nc.scalar.dma_start(
                out=wdkv_sb[:, g * 2 : (g + 1) * 2, :],
                in_=wdkv.ap()[:, g * 2 : (g + 1) * 2],
            )
        cos_sb = wpool.tile([32, T], B16)
        nc.sync.dma_start(out=cos_sb[:], in_=cosT.ap())
        sin_sb = wpool.tile([32, T], B16)
        nc.sync.dma_start(out=sin_sb[:], in_=sinT.ap())

        # per-head projection weights (needed from the q-projection phase)
        a_sbs, wqr_sbs, bt_sbs = [], [], []
        for h in range(HPC):
            a_sb = hwpool.tile([128, 4, HS], B16, tag="a_sb", bufs=4, name=f"a{h}")
            nc.sync.dma_start(out=a_sb[:], in_=A.ap()[h])
            bt_sb = hwpool.tile([128, 4, HS], B16, tag="bt_sb", bufs=4, name=f"bt{h}")
            nc.scalar.dma_start(out=bt_sb[:], in_=BT.ap()[h])
            a_sbs.append(a_sb)
            bt_sbs.append(bt_sb)
        for p in range(HPC // 2):
            wqr_sb = hwpool.tile(
                [128, 4, 2 * DHR], B16, tag="wqr_sb", bufs=2, name=f"wq{p}"
            )
            nc.sync.dma_start(out=wqr_sb[:], in_=wqr.ap()[p])
            wqr_sbs.append(wqr_sb)

        def xsl_at(ct):
            return halves[ct // 8][:, ct % 8, :]

        # c_q chunk first — its gather has the earliest consumer (the q
        # projections).  Staged in cq_sb's chunk-0 region: the bounce DMA
        # reads it before the gather-backs overwrite all chunks.
        for qt in range(4):
            ps = psW.tile([128, 512], F32, tag="psw")
            for ct in range(16):
                nc.tensor.matmul(
                    ps[:],
                    wdq_sb[:, ct, qt * 128 : (qt + 1) * 128],
                    xsl_at(ct),
                    start=(ct == 0),
                    stop=(ct == 15),
                )
            nc.scalar.activation(cq_sb[:, qt, 0:512], ps[:], Copy)
        nc.gpsimd.dma_start(out=cq_bi[:], in_=cq_sb[:, :, 0:512])
        nc.gpsimd.collective_compute(
            "AllGather",
            mybir.AluOpType.bypass,
            replica_groups=GROUPS,
            ins=[cq_bi[:].opt()],
            outs=[cq_bo[:].opt()],
        )

        # k_r chunk (tiny gather, unblocks kcat)
        kr_loc = cpool.tile([64, 512], B16)
        ps = psW.tile([128, 512], F32, tag="psw")
        for ct in range(16):
            nc.tensor.matmul(
                ps[0:64, :],
                wkr_sb[:, ct, :],
                xsl_at(ct),
                start=(ct == 0),
                stop=(ct == 15),
            )
        _rope(
            nc,
            tpool,
            ps[0:32, :],
            ps[32:64, :],
            kr_loc[0:32, :],
            kr_loc[32:64, :],
            cosk_sb[:],
            sink_sb[:],
        )
        nc.scalar.activation(krh_sb[0:64, 0:512], kr_loc[:], Copy)
        lo_bf = tpool.tile([64, 512], B16, tag="krlo", bufs=1)
        nc.vector.tensor_sub(lo_bf[:], kr_loc[:], krh_sb[0:64, 0:512])
        nc.scalar.activation(krh_sb[64:128, 0:512], lo_bf[:], Copy, scale=4.0)
        nc.gpsimd.dma_start(out=krh_bi[:], in_=krh_sb[:, 0:512])
        nc.gpsimd.collective_compute(
            "AllGather",
            mybir.AluOpType.bypass,
            replica_groups=GROUPS,
            ins=[krh_bi[:].opt()],
            outs=[krh_bo[:].opt()],
        )

        # c_kv chunk (own quarter, gathered; its consumers run late).
        # Staged in ckv_sb's chunk-0 region; gather-backs overwrite it.
        for kt in range(4):
            ps = psW.tile([128, 512], F32, tag="psw")
            for ct in range(16):
                nc.tensor.matmul(
                    ps[:],
                    wdkv_sb[:, ct, kt * 128 : (kt + 1) * 128],
                    xsl_at(ct),
                    start=(ct == 0),
                    stop=(ct == 15),
                )
            nc.vector.tensor_copy(ckv_sb[:, kt, 0:512], ps[:])
        nc.gpsimd.dma_start(out=ckv_bi[:], in_=ckv_sb[:, :, 0:512])
        nc.gpsimd.collective_compute(
            "AllGather",
            mybir.AluOpType.bypass,
            replica_groups=GROUPS,
            ins=[ckv_bi[:].opt()],
            outs=[ckv_bo[:].opt()],
        )

        # mid-needed big weights: wuv reuses wdkv's buffer (released just
        # above); issued here so the buffer-reuse wait doesn't block
        # earlier scalar work
        wuv_sb = wpool.tile([128, 16, NLKV], B16, tag="wkv", bufs=1, name="wuv")
        for g in range(4):
            nc.scalar.dma_start(
                out=wuv_sb[:, g * 4 : (g + 1) * 4, :],
                in_=wuv.ap()[:, g * 4 : (g + 1) * 4],
            )
        woh_sb = wpool.tile([128, 16, HPC * HS], B16)
        for g in range(4):
            nc.scalar.dma_start(
                out=woh_sb[:, g * 4 : (g + 1) * 4, :],
                in_=woT.ap()[:, g * 4 : (g + 1) * 4],
            )
        mask_sb = wpool.tile([128, 896], B16)
        nc.scalar.dma_start(out=mask_sb[:], in_=masks.ap())

        # gather-backs (gpsimd: it is the comms engine and has nothing
        # else pending, so its post-AG stall is free)
        for j in range(NT):
            nc.gpsimd.dma_start(
                out=cq_sb[:, :, j * 512 : (j + 1) * 512], in_=cq_bo[:][j]
            )
        for j in range(NT):
            nc.gpsimd.dma_start(
                out=krh_sb[:, j * 512 : (j + 1) * 512], in_=krh_bo[:][j]
            )
        for j in range(NT):
            nc.gpsimd.dma_start(
                out=ckv_sb[:, :, j * 512 : (j + 1) * 512], in_=ckv_bo[:][j]
            )

        # ---- shared V phase: all 4 heads at once (N=512 matmuls) ----
        # veff_all[k, (h,d)] = sum_c W_uv[c,k] * W_o[h-block].T[c,d]
        veff_sb = cpool.tile([128, 4, HPC * HS], B16)
        for kt in range(4):
            ps = psW.tile([128, 512], F32, tag="psw")
            for ct in range(16):
                nc.tensor.matmul(
                    ps[:],
                    wuv_sb[:, ct, kt * 128 : (kt + 1) * 128],
                    woh_sb[:, ct, :],
                    start=(ct == 0),
                    stop=(ct == 15),
                )
            nc.vector.tensor_copy(veff_sb[:, kt, :], ps[:])

        # ---- Q projections, heads processed in pairs (one M=128 rope
        # matmul covers both heads' W_qr) ----
        # fp8 DoubleRow operands: slot 0 = content (128d), slot 1 =
        # rope.  K side: (kr_hi | 4*kr_lo); Q side: (qr | qr/4) so the
        # rope product is qr*kr_hi + (qr/4)*4*kr_lo = qr*kr exactly
        # in kr.
        qcats = [
            hpool.tile([128, 2, T], F8A, tag="qh", bufs=4, name=f"qcat{h}")
            for h in range(HPC)
        ]
        for p in range(HPC // 2):
            for j in range(NT):
                t0 = j * 512
                for h in (2 * p, 2 * p + 1):
                    ps = psW.tile([128, 512], F32, tag="psw")
                    for qt in range(4):
                        nc.tensor.matmul(
                            ps[:],
                            a_sbs[h][:, qt, :],
                            cq_sb[:, qt, t0 : t0 + 512],
                            start=(qt == 0),
                            stop=(qt == 3),
                        )
                    nc.scalar.activation(
                        qcats[h][:, 0, t0 : t0 + 512], ps[:], Copy
                    )
                ps = psW.tile([128, 512], F32, tag="psw")
                for qt in range(4):
                    nc.tensor.matmul(
                        ps[:],
                        wqr_sbs[p][:, qt, :],
                        cq_sb[:, qt, t0 : t0 + 512],
                        start=(qt == 0),
                        stop=(qt == 3),
                    )
                for u, h in enumerate((2 * p, 2 * p + 1)):
                    qcat_sb = qcats[h]
                    _rope(
                        nc,
                        tpool,
                        ps[64 * u : 64 * u + 32, :],
                        ps[64 * u + 32 : 64 * u + 64, :],
                        qcat_sb[0:32, 1, t0 : t0 + 512],
                        qcat_sb[32:64, 1, t0 : t0 + 512],
                        cos_sb[:, t0 : t0 + 512],
                        sin_sb[:, t0 : t0 + 512],
                        stage_re=(u == 1),
                    )
                    nc.scalar.activation(
                        qcat_sb[64:128, 1, t0 : t0 + 512],
                        qcat_sb[0:64, 1, t0 : t0 + 512],
                        Copy,
                        scale=0.25,
                    )

        # v_aug[s, (h, d|1)]: v for all heads + ones column per head
        vaug_sb = cpool.tile([128, NS, HPC, 129], B16)
        for st in range(NS):
            ps = psW.tile([128, 512], F32, tag="psw")
            for kt in range(4):
                nc.tensor.matmul(
                    ps[:],
                    ckv_sb[:, kt, st * 128 : (st + 1) * 128],
                    veff_sb[:, kt, :],
                    start=(kt == 0),
                    stop=(kt == 3),
                )
            nc.vector.tensor_copy(
                vaug_sb[:, st, :, 0:128],
                ps[:].rearrange("p (h d) -> p h d", h=HPC),
            )
            nc.vector.memset(vaug_sb[:, st, :, 128:129], 1.0)

        # ---- per-head: K projection + attention ----
        for h in range(HPC):
            qcat_sb = qcats[h]
            kcat_sb = hpool.tile([128, 2, T], F8A, tag="kh")
            for j in range(NT):
                t0 = j * 512
                nc.vector.tensor_copy(
                    kcat_sb[:, 1, t0 : t0 + 512], krh_sb[:, t0 : t0 + 512]
                )
                ps = psW.tile([128, 512], F32, tag="psw")
                for kt in range(4):
                    nc.tensor.matmul(
                        ps[:],
                        bt_sbs[h][:, kt, :],
                        ckv_sb[:, kt, t0 : t0 + 512],
                        start=(kt == 0),
                        stop=(kt == 3),
                    )
                nc.vector.tensor_copy(kcat_sb[:, 0, t0 : t0 + 512], ps[:])

            # attention: for each t-chunk, accumulate over causal s-tiles.
            # S/exp runs PV_LAG s-tiles ahead of PV so the PE stream does
            # not stall on psY slot release at chunk boundaries.
            for j in range(NT):
                t0 = j * 512
                ys = [
                    psY.tile([128, 132], F32, tag="psy", name=f"psy_{h}_{j}_{m}")
                    for m in range(4)
                ]
                n_st = 4 * j + 4
                pts = {}

                def s_exp(i, h=h, j=j, t0=t0):
                    ss = psW.tile([128, 512], F32, tag="psw")
                    nc.tensor.matmul(
                        ss[:],
                        kcat_sb[:, :, i * 128 : (i + 1) * 128],
                        qcat_sb[:, :, t0 : t0 + 512],
                        start=True,
                        stop=True,
                        perf_mode=DR,
                    )
                    pt = ppool.tile([128, 512], B16, tag="pt", name=f"pt_{h}_{j}_{i}")
                    m2 = i - 4 * j
                    if m2 <= 0:
                        nc.scalar.activation(pt[:], ss[:], Exp, scale=SCALE)
                        if m2 == 0:
                            nc.vector.tensor_mul(
                                pt[:], pt[:], mask_sb[:, 384:896]
                            )
                    else:
                        # band tile: PV only reads cols >= 128*m2; exp/mask
                        # just that suffix (the prefix is never consumed)
                        lo = 128 * m2
                        nc.scalar.activation(
                            pt[:, lo:512], ss[:, lo:512], Exp, scale=SCALE
                        )
                        nc.vector.tensor_mul(
                            pt[:, lo:512],
                            pt[:, lo:512],
                            mask_sb[:, 384 : 896 - lo],
                        )
                    pts[i] = pt

                def pv(i, ys=ys, pts=pts, j=j):
                    mm0 = max(0, i - 4 * j)
                    for m in range(mm0, 4):
                        nc.tensor.matmul(
                            ys[m][:, 0:129],
                            pts[i][:, m * 128 : (m + 1) * 128],
                            vaug_sb[:, i, h, 0:129],
                            start=(i == 0),
                            stop=(i == 4 * j + m),
                        )

                for i in range(n_st):
                    s_exp(i)
                    if i >= PV_LAG:
                        pv(i - PV_LAG)
                for i in range(max(0, n_st - PV_LAG), n_st):
                    pv(i)

                for m in range(4):
                    recip = tpool.tile([128, 1], F32, tag="recip")
                    nc.vector.reciprocal(recip[:], ys[m][:, 128:129])
                    o_sb = opool.tile([128, HS], F32, tag="o_sb")
                    nc.vector.tensor_scalar_mul(o_sb[:], ys[m][:, 0:128], recip[:])
                    nc.scalar.dma_start(
                        out=out.ap()[
                            t0 + m * 128 : t0 + (m + 1) * 128,
                            h * HS : (h + 1) * HS,
                        ],
                        in_=o_sb[:],
                    )

    nc.compile()
    return nc


def _rope(nc, tpool, ps_re, ps_im, dst_re, dst_im, cs, sn, stage_re=False):
    """ps_re/ps_im are 32-partition PSUM views of the projected (re|im
    col-permuted) halves.  Write roped values into dst_re (partitions
    0:32 of the target) and dst_im (32:64).  DVE tensor_tensor requires
    all operands at the same start partition, so halves that don't start
    at partition 0 are staged through base-0 tiles with ACT copies (ACT
    allows cross-base)."""
    if stage_re:
        re_sb = tpool.tile([32, 512], B16, tag="resrc", bufs=2)
        nc.scalar.activation(re_sb[:], ps_re, Copy)
        ps_re = re_sb[:]
    im_sb = tpool.tile([32, 512], B16, tag="imsrc", bufs=2)
    nc.scalar.activation(im_sb[:], ps_im, Copy)
    t1 = tpool.tile([32, 512], B16, tag="ropet1", bufs=2)
    t2 = tpool.tile([32, 512], B16, tag="ropet2", bufs=2)
    nc.vector.tensor_mul(t1[:], ps_re, cs)
    nc.vector.tensor_mul(t2[:], im_sb[:], sn)
    nc.vector.tensor_sub(dst_re, t1[:], t2[:])
    t3 = tpool.tile([32, 512], B16, tag="ropet1", bufs=2, name="t3")
    t4 = tpool.tile([32, 512], B16, tag="ropet2", bufs=2, name="t4")
    nc.vector.tensor_mul(t3[:], ps_re, sn)
    nc.vector.tensor_mul(t4[:], im_sb[:], cs)
    im_ro = tpool.tile([32, 512], B16, tag="imrope", bufs=1)
    nc.vector.tensor_add(im_ro[:], t3[:], t4[:])
    nc.scalar.activation(dst_im, im_ro[:], Copy)


_NC_CACHE = {}


def _get_nc():
    if "nc" not in _NC_CACHE:
        _NC_CACHE["nc"] = build()
    return _NC_CACHE["nc"]


def _part_major(a, pt=128):
    """[pt*n, ...] -> [128, n, ...] partition-major contiguous."""
    n = a.shape[0] // pt
    return np.ascontiguousarray(
        a.reshape(n, pt, *a.shape[1:]).transpose(1, 0, *range(2, a.ndim + 1))
    )


def _prep_in_maps(x, cos, sin, W_dq, W_uq, W_dkv, W_uk, W_uv, W_qr, W_kr, W_o):
    perm = np.concatenate([np.arange(0, DHR, 2), np.arange(1, DHR, 2)])

    shared = {
        "wdq": _part_major(np.ascontiguousarray(W_dq.T).astype(bf16)),
        "wdkv": _part_major(np.ascontiguousarray(W_dkv.T).astype(bf16)),
        "wkr": _part_major(np.ascontiguousarray(W_kr.T[:, perm]).astype(bf16)),
        "wuv": _part_major(np.ascontiguousarray(W_uv).astype(bf16)),
        "cosT": np.ascontiguousarray(cos.T).astype(bf16),
        "sinT": np.ascontiguousarray(sin.T).astype(bf16),
    }
    p = np.arange(128)[:, None]
    u = np.arange(896)[None, :]
    shared["masks"] = np.ascontiguousarray((p <= u - 384).astype(bf16))  # [128, 896]

    # x[b] -> xp[p, j, ct, f] = x[b][j*512+f, ct*128+p]
    xps = []
    for b in range(B):
        xb = np.asarray(x[b]).astype(bf16)  # [T, C]
        xps.append(
            np.ascontiguousarray(xb.reshape(NT, 512, 16, 128).transpose(3, 0, 2, 1))
        )

    A_full = np.asarray(W_uq).reshape(NLQ, NH, HS)
    B_full = np.asarray(W_uk).reshape(NH, HS, NLKV)

    head_maps = []
    for g in range(4):
        hs = [4 * g + i for i in range(HPC)]
        # SCALE is applied in the Exp activation on-chip (keeps the fp8
        # q/k operands in a healthy range)
        A_np = np.stack([_part_major(A_full[:, h, :].astype(bf16)) for h in hs])
        BT_np = np.stack([_part_major(np.ascontiguousarray(B_full[h].T).astype(bf16)) for h in hs])
        # head-pair packed: [pair, 128, 4, 2*DHR], cols 0:64 = even head,
        # 64:128 = odd head
        wqr_heads = [
            _part_major(
                np.ascontiguousarray(
                    W_qr[h * DHR : (h + 1) * DHR, :].T[:, perm]
                ).astype(bf16)
            )
            for h in hs
        ]
        wqr_np = np.stack(
            [
                np.concatenate([wqr_heads[2 * p], wqr_heads[2 * p + 1]], axis=2)
                for p in range(HPC // 2)
            ]
        )
        # [128(c p), 16(ct), HPC*HS] with free = (h, d)
        woT_np = np.stack(
            [
                _part_major(np.ascontiguousarray(W_o[h * HS : (h + 1) * HS, :].T).astype(bf16))
                for h in hs
            ],
            axis=2,
        ).reshape(128, 16, HPC * HS)
        head_maps.append(
            {
                "A": np.ascontiguousarray(A_np),
                "BT": np.ascontiguousarray(BT_np),
                "wqr": np.ascontiguousarray(wqr_np),
                "woT": np.ascontiguousarray(woT_np),
            }
        )

    cosT_np = shared["cosT"]
    sinT_np = shared["sinT"]
    in_maps = []
    for core in range(NCORES):
        b, g = core // 4, core % 4
        im = dict(shared)
        # own T-chunk of x for the distributed prologue, plus the matching
        # cos/sin slice for the local k_r rope
        im["xpo"] = np.ascontiguousarray(xps[b][:, g])
        im["cosk"] = np.ascontiguousarray(cosT_np[:, g * 512 : (g + 1) * 512])
        im["sink"] = np.ascontiguousarray(sinT_np[:, g * 512 : (g + 1) * 512])
        im.update(head_maps[g])
        in_maps.append(im)
    return in_maps


def kernel_run(inputs, trace=False, trace_kwargs=None):
    nc = _get_nc()
    in_maps = _prep_in_maps(**{k: np.asarray(v) for k, v in inputs.items()})
    res = run_bass_kernel_spmd(
        nc,
        in_maps,
        core_ids=list(range(NCORES)),
        trace=trace,
        **(trace_kwargs or {}),
    )
    y = np.empty((B, T, C), np.float32)
    for core in range(NCORES):
        b, g = core // 4, core % 4
        y[b][:, g * 512 : (g + 1) * 512] = res.results[core]["out"]
    return y, res


def kernel(**inputs):
    y, _ = kernel_run(inputs)
    return y

